# revision 24
# baseline (speedup 1.0000x reference)
"""Social-LSTM single-step kernel for 8 Trainium2 NeuronCores.

Sort pedestrians by x on the host; shard sorted targets across 8 cores
(128 each). Grid neighbors lie within +-0.2 in x, so each core gets a
host-sliced window of W sorted neighbors. On chip, each core computes
per-pair grid-cell codes, expands them to bf16 one-hot masks against a
code ramp (wide is_equal, 2x DVE mode), and accumulates
social^T[h, n] per grid cell in PSUM via TensorE matmuls with the
neighbor hidden states stationary. Social pooling, embedding, LSTM and
the output projection run on-chip; the host only permutes/slices
inputs and inverse-permutes the output shards.
"""
import numpy as np
import ml_dtypes

from concourse import bass, mybir
from concourse.tile import TileContext, ScopedClock
from concourse.bass_utils import run_bass_kernel_spmd

F32 = mybir.dt.float32
I32 = mybir.dt.int32
BF16 = mybir.dt.bfloat16
ALU = mybir.AluOpType
ACT = mybir.ActivationFunctionType
BF = ml_dtypes.bfloat16

N = 1024
RNN = 128
EMB = 64
GS = 8
G = GS * GS
NMIX = 20
NCORE = 8
NC_CHUNK = N // NCORE
MDT = BF16
MNP = BF
RCH = 4                    # ramp/mask column chunks
RC_G = G // RCH            # 16 cells per ramp chunk
RC_W = RC_G * NC_CHUNK     # 2048 mask columns per chunk
PSG = [12, 12, 12, 12, 12, 4]   # psum group sizes (cells)


def _patched_drain(self, tick_clock, wait_clock):
    nop_inst = self.nc.sync.nop()
    wait_clock.add_sem_waits(nop_inst.ins, ScopedClock({None: tick_clock.global_clock}))
    si = nop_inst.ins.sync_info
    waits = list(si.on_wait or [])
    si.on_wait = waits[:1]
    for i in range(1, len(waits)):
        extra = self.nc.sync.nop()
        extra.ins.sync_info = mybir.SyncInfo(on_update=[], on_wait=[waits[i]])
    self.nc.sync.drain()
    popped = self.nc._tile_sem_poison_stack.pop()
    assert popped is self._sem_poison
    # The Bass preamble re-clears all kernel sems at the next execution's
    # start, so the exit-time clear instructions are redundant; keep only
    # the bookkeeping (free-pool + poison sets).
    sems = list(self.sems.allocated().values())
    sem_nums = [s.num for s in sems]
    self.nc._state.prepend_free_semaphores(sem_nums)
    for poison_set in self.nc._tile_sem_poison_stack:
        poison_set.update(sem_nums)


TileContext._drain_and_barrier = _patched_drain


def _split_multi_waits(nc):
    for fn in nc.m.functions:
        for bb in fn.blocks:
            new_insts = []
            for inst in bb.instructions:
                si = getattr(inst, "sync_info", None)
                waits = list(si.on_wait) if si is not None and si.on_wait else []
                if len(waits) > 1:
                    for w in waits[:-1]:
                        new_insts.append(mybir.InstNoOp(
                            name=nc.get_next_instruction_name(), ins=[], outs=[],
                            engine=inst.engine,
                            sync_info=mybir.SyncInfo(on_update=[], on_wait=[w]),
                        ))
                    si.on_wait = [waits[-1]]
                new_insts.append(inst)
            bb.instructions = new_insts


def _build_program(wc):
    W = wc * 128
    nc = bass.Bass(target_bir_lowering=False)

    xabs_r = nc.dram_tensor("xabs_r", [128, 2 * wc], F32, kind="ExternalInput")
    xnb = nc.dram_tensor("xnb", [128, NC_CHUNK], F32, kind="ExternalInput")
    ynb = nc.dram_tensor("ynb", [128, NC_CHUNK], F32, kind="ExternalInput")
    actc = nc.dram_tensor("actc", [128, 2], F32, kind="ExternalInput")
    eye_r = nc.dram_tensor("eye_r", [128, W], MDT, kind="ExternalInput")
    ramp_in = nc.dram_tensor("ramp_in", [RCH * 128, RC_W], MDT, kind="ExternalInput")
    h_winp = nc.dram_tensor("h_winp", [128, W], MDT, kind="ExternalInput")
    wsoc_r = nc.dram_tensor("wsoc_r", [RNN, G * EMB], MDT, kind="ExternalInput")
    wembT = nc.dram_tensor("wembT", [2, EMB], F32, kind="ExternalInput")
    xoffT = nc.dram_tensor("xoffT", [2, NC_CHUNK], F32, kind="ExternalInput")
    b_embsoc = nc.dram_tensor("b_embsoc", [128, 1], F32, kind="ExternalInput")
    wihT = nc.dram_tensor("wihT", [128, 4 * RNN], F32, kind="ExternalInput")
    whhT = nc.dram_tensor("whhT", [RNN, 4 * RNN], F32, kind="ExternalInput")
    bgates_ih = nc.dram_tensor("bgates_ih", [128, 4], F32, kind="ExternalInput")
    bgates_hh = nc.dram_tensor("bgates_hh", [128, 4], F32, kind="ExternalInput")
    hT_c = nc.dram_tensor("hT_c", [RNN, NC_CHUNK], F32, kind="ExternalInput")
    cT_c = nc.dram_tensor("cT_c", [RNN, NC_CHUNK], F32, kind="ExternalInput")
    woutT = nc.dram_tensor("woutT", [RNN, 6 * NMIX], F32, kind="ExternalInput")
    bout = nc.dram_tensor("bout", [6 * NMIX, 1], F32, kind="ExternalInput")
    outT = nc.dram_tensor("outT", [6 * NMIX, NC_CHUNK], F32, kind="ExternalOutput")

    with TileContext(nc) as tc:
        with (
            tc.tile_pool(name="const", bufs=1) as cpool,
            tc.tile_pool(name="masks", bufs=1) as maskpool,
            tc.tile_pool(name="soc", bufs=2) as socpool,
            tc.tile_pool(name="work", bufs=2) as work,
            tc.tile_pool(name="psum", bufs=1, space="PSUM") as pp,
            tc.tile_pool(name="psum_soc", bufs=2, space="PSUM") as pps,
        ):
            # ---- small / latency-critical inputs on the sync queue ----
            xabs_sb = cpool.tile([128, 2 * wc], F32, tag="xabs")
            nc.sync.dma_start(xabs_sb[:, :], xabs_r[:, :])
            xnb_sb = cpool.tile([128, NC_CHUNK], F32, tag="xnb")
            nc.sync.dma_start(xnb_sb[:, :], xnb[:, :])
            ynb_sb = cpool.tile([128, NC_CHUNK], F32, tag="ynb")
            nc.sync.dma_start(ynb_sb[:, :], ynb[:, :])
            actc_sb = cpool.tile([128, 2], F32, tag="actc")
            nc.sync.dma_start(actc_sb[:, :], actc[:, :])
            # xm20[m] = (x_m + 0.2) * 20 bias columns for the ACT Identity
            # form v = xnb*(-20) + xm20 (bit-identical bins, HW verified)
            xm20 = cpool.tile([128, 2 * wc], F32, tag="xm20")
            nc.vector.tensor_scalar(xm20[:, :], xabs_sb[:, :], 0.2, 20.0,
                                    op0=ALU.add, op1=ALU.mult)
            # DMA schedule by need-by time. gpsimd DMA is SWDGE on the POOL
            # engine which contends with DVE for the shared SBUF port — keep
            # it empty. sync carries the critical stream in need order;
            # scalar (free after its table load) carries the rest.
            eye_sb = cpool.tile([128, W], MDT, tag="eye")
            nc.sync.dma_start(eye_sb[:, :], eye_r[:, :])
            h_big = cpool.tile([128, W], MDT, tag="h_big")
            nc.sync.dma_start(h_big[:, :], h_winp[:, :])
            ramp = []
            for c in range(RCH):
                ramp_t = cpool.tile([128, RC_W], MDT, tag=f"ramp{c}")
                ramp.append(ramp_t)
            nc.sync.dma_start(ramp[0][:, :], ramp_in[0:128, :])
            nc.sync.dma_start(ramp[1][:, :], ramp_in[128:256, :])
            wsoc_sb = cpool.tile([RNN, G * EMB], MDT, tag="wsoc")
            nc.scalar.dma_start(wsoc_sb[:, :], wsoc_r[:, :])
            nc.sync.dma_start(ramp[2][:, :], ramp_in[256:384, :])
            nc.sync.dma_start(ramp[3][:, :], ramp_in[384:512, :])
            wihT_sb = cpool.tile([128, 4 * RNN], F32, tag="wihT")
            nc.scalar.dma_start(wihT_sb[:, :], wihT[:, :])
            whhT_sb = cpool.tile([RNN, 4 * RNN], F32, tag="whhT")
            nc.scalar.dma_start(whhT_sb[:, :], whhT[:, :])
            woutT_sb = cpool.tile([RNN, 6 * NMIX], F32, tag="woutT")
            nc.scalar.dma_start(woutT_sb[:, :], woutT[:, :])
            wembT_sb = cpool.tile([2, EMB], F32, tag="wembT")
            nc.sync.dma_start(wembT_sb[:, :], wembT[:, :])
            xoffT_sb = cpool.tile([2, NC_CHUNK], F32, tag="xoffT")
            nc.sync.dma_start(xoffT_sb[:, :], xoffT[:, :])
            b_es_sb = cpool.tile([128, 1], F32, tag="b_embsoc")
            nc.sync.dma_start(b_es_sb[:, :], b_embsoc[:, :])
            hT_sb = cpool.tile([RNN, NC_CHUNK], F32, tag="hT")
            nc.sync.dma_start(hT_sb[:, :], hT_c[:, :])
            cT_sb = cpool.tile([RNN, NC_CHUNK], F32, tag="cT")
            nc.sync.dma_start(cT_sb[:, :], cT_c[:, :])
            bgi_sb = cpool.tile([128, 4], F32, tag="bgates_ih")
            nc.sync.dma_start(bgi_sb[:, :], bgates_ih[:, :])
            bgh_sb = cpool.tile([128, 4], F32, tag="bgates_hh")
            nc.sync.dma_start(bgh_sb[:, :], bgates_hh[:, :])
            bout_sb = cpool.tile([6 * NMIX, 1], F32, tag="bout")
            nc.sync.dma_start(bout_sb[:, :], bout[:, :])

            # ---- cell codes per neighbor chunk ----
            # code = 108 - t2x - 11*t2y,  t2 = rint(relu(9 - relu(v + 0.5)))
            # (ACT converts f32->i32 round-to-nearest-even; HW verified)
            cells = []
            for mc in range(wc):
                vx = work.tile([128, NC_CHUNK], F32, tag="vx")
                nc.scalar.activation(vx[:, :], xnb_sb[:, :], ACT.Identity,
                                     bias=xm20[:, 2 * mc:2 * mc + 1],
                                     scale=-20.0)
                vy = work.tile([128, NC_CHUNK], F32, tag="vy")
                nc.scalar.activation(vy[:, :], ynb_sb[:, :], ACT.Identity,
                                     bias=xm20[:, 2 * mc + 1:2 * mc + 2],
                                     scale=-20.0)
                t2x = work.tile([128, NC_CHUNK], I32, tag="t2x")
                t2y = work.tile([128, NC_CHUNK], I32, tag="t2y")
                t1x = work.tile([128, NC_CHUNK], F32, tag="t1x")
                nc.scalar.activation(t1x[:, :], vx[:, :], ACT.Relu,
                                     bias=actc_sb[:, 0:1], scale=1.0)
                nc.scalar.activation(t2x[:, :], t1x[:, :], ACT.Relu,
                                     bias=actc_sb[:, 1:2], scale=-1.0)
                t1y = work.tile([128, NC_CHUNK], F32, tag="t1y")
                nc.scalar.activation(t1y[:, :], vy[:, :], ACT.Relu,
                                     bias=actc_sb[:, 0:1], scale=1.0)
                nc.scalar.activation(t2y[:, :], t1y[:, :], ACT.Relu,
                                     bias=actc_sb[:, 1:2], scale=-1.0)
                u = work.tile([128, NC_CHUNK], I32, tag="u")
                nc.vector.tensor_scalar(u[:, :], t2y[:, :], -11, 108,
                                        op0=ALU.mult, op1=ALU.add)
                cc = work.tile([128, NC_CHUNK], MDT, tag=f"cell{mc}")
                nc.vector.tensor_tensor(cc[:, :], u[:, :], t2x[:, :],
                                        op=ALU.subtract)
                nc.vector.tensor_tensor(cc[:, :], cc[:, :],
                                        eye_sb[:, mc * 128:(mc + 1) * 128],
                                        op=ALU.add)
                cells.append(cc)

            # ---- masks: chunk-major so PE group g can start early ----
            masks = {}
            for c in range(RCH):
                for mc in range(wc):
                    m = maskpool.tile([128, RC_W], MDT, tag=f"m{mc}g{c}")
                    cb = cells[mc][:, :].unsqueeze(1).broadcast_to(
                        [128, RC_G, NC_CHUNK])
                    nc.vector.tensor_tensor(m[:, :], cb, ramp[c][:, :],
                                            op=ALU.is_equal)
                    masks[(mc, c)] = m

            # ---- social matmuls + pooling, double-buffered psum groups ----
            xin_ps = pp.tile([128, NC_CHUNK], F32, tag="xin_ps")
            g0 = 0
            for gi, gsz in enumerate(PSG):
                soc_ps = pps.tile([128, 12 * NC_CHUNK], F32, tag="soc_ps")
                for mc in range(wc):
                    for s in range(gsz // 4):
                        cell0 = g0 + s * 4
                        c = cell0 // RC_G
                        off = (cell0 % RC_G) * NC_CHUNK
                        nc.tensor.matmul(
                            soc_ps[:, s * 512:(s + 1) * 512],
                            h_big[:, mc * 128:(mc + 1) * 128],
                            masks[(mc, c)][:, off:off + 512],
                            start=(mc == 0), stop=(mc == wc - 1))
                soc_sb = socpool.tile([128, 12 * NC_CHUNK], MDT, tag="soc_sb")
                nc.scalar.activation(soc_sb[:, :gsz * NC_CHUNK],
                                     soc_ps[:, :gsz * NC_CHUNK], ACT.Copy,
                                     bias=0.0, scale=1.0)
                for gl in range(gsz):
                    g = g0 + gl
                    nc.tensor.matmul(xin_ps[EMB:, :],
                                     wsoc_sb[:, g * EMB:(g + 1) * EMB],
                                     soc_sb[:, gl * NC_CHUNK:(gl + 1) * NC_CHUNK],
                                     start=(g == 0), stop=(g == G - 1))
                g0 += gsz

            # ---- embedding ----
            nc.tensor.matmul(xin_ps[:EMB, :], wembT_sb[:, :], xoffT_sb[:, :],
                             start=True, stop=True)
            xinT = work.tile([128, NC_CHUNK], F32, tag="xinT")
            nc.scalar.activation(xinT[:, :], xin_ps[:, :], ACT.Relu,
                                 bias=b_es_sb[:, 0:1], scale=1.0)

            # ---- LSTM ----
            bg_sb = cpool.tile([128, 4], F32, tag="bgates")
            nc.vector.tensor_tensor(bg_sb[:, :], bgi_sb[:, :], bgh_sb[:, :],
                                    op=ALU.add)
            acts = []
            for q in range(4):
                g_ps = pp.tile([128, NC_CHUNK], F32, tag="g_ps")
                nc.tensor.matmul(g_ps[:, :], wihT_sb[:, q * RNN:(q + 1) * RNN],
                                 xinT[:, :], start=True, stop=False)
                nc.tensor.matmul(g_ps[:, :], whhT_sb[:, q * RNN:(q + 1) * RNN],
                                 hT_sb[:, :], start=False, stop=True)
                gq = work.tile([128, NC_CHUNK], F32, tag=f"gate{q}")
                func = ACT.Tanh if q == 2 else ACT.Sigmoid
                nc.scalar.activation(gq[:, :], g_ps[:, :], func,
                                     bias=bg_sb[:, q:q + 1], scale=1.0)
                acts.append(gq)

            fc = work.tile([128, NC_CHUNK], F32, tag="fc")
            nc.vector.tensor_tensor(fc[:, :], acts[1][:, :], cT_sb[:, :],
                                    op=ALU.mult)
            ig = work.tile([128, NC_CHUNK], F32, tag="ig")
            nc.vector.tensor_tensor(ig[:, :], acts[0][:, :], acts[2][:, :],
                                    op=ALU.mult)
            cnew = work.tile([128, NC_CHUNK], F32, tag="cnew")
            nc.vector.tensor_tensor(cnew[:, :], fc[:, :], ig[:, :], op=ALU.add)
            tc_t = work.tile([128, NC_CHUNK], F32, tag="tc")
            nc.scalar.activation(tc_t[:, :], cnew[:, :], ACT.Tanh,
                                 bias=0.0, scale=1.0)
            hn = work.tile([128, NC_CHUNK], F32, tag="hn")
            nc.vector.tensor_tensor(hn[:, :], acts[3][:, :], tc_t[:, :],
                                    op=ALU.mult)

            # ---- output projection ----
            out_ps = pp.tile([6 * NMIX, NC_CHUNK], F32, tag="g_ps")
            nc.tensor.matmul(out_ps[:, :], woutT_sb[:, :], hn[:, :],
                             start=True, stop=True)
            outT_sb = work.tile([6 * NMIX, NC_CHUNK], F32, tag="outT")
            nc.vector.tensor_scalar(outT_sb[:, :], out_ps[:, :],
                                    bout_sb[:, 0:1], None, op0=ALU.add)
            nc.sync.dma_start(outT[:, :], outT_sb[:, :])

    _split_multi_waits(nc)
    return nc


_NC_CACHE = {}


def _get_program(wc):
    if wc not in _NC_CACHE:
        _NC_CACHE[wc] = _build_program(wc)
    return _NC_CACHE[wc]


def _make_ramp():
    gy, gx, n = np.meshgrid(np.arange(GS), np.arange(GS), np.arange(NC_CHUNK),
                            indexing="ij")
    vals = (12 + gx + 11 * gy).reshape(1, G * NC_CHUNK)
    full = np.broadcast_to(vals, (128, G * NC_CHUNK)).astype(MNP)
    # chunked layout: [RCH*128, RC_W], chunk c = rows 128c..128c+127
    return np.ascontiguousarray(
        full.reshape(128, RCH, RC_W).transpose(1, 0, 2).reshape(RCH * 128, RC_W))


def _prep_inputs(xoff, xabs, h0, c0, W_emb, b_emb, W_soc, b_soc,
                 W_ih, W_hh, b_ih, b_hh, W_out, b_out):
    f32 = np.float32
    xoff = np.asarray(xoff, f32)
    xabs = np.asarray(xabs, f32)
    h = np.asarray(h0, f32)[0]
    c = np.asarray(c0, f32)[0]
    W_emb = np.asarray(W_emb, f32)
    W_soc = np.asarray(W_soc, f32)
    W_ih = np.asarray(W_ih, f32)
    W_hh = np.asarray(W_hh, f32)
    W_out = np.asarray(W_out, f32)

    perm = np.argsort(xabs[:, 0], kind="stable")
    xs = xabs[perm]
    xoff_s = xoff[perm]
    h_s = h[perm]
    c_s = c[perm]

    los, his = [], []
    for k in range(NCORE):
        ch = xs[k * NC_CHUNK:(k + 1) * NC_CHUNK, 0]
        los.append(np.searchsorted(xs[:, 0], ch.min() - f32(0.21), "left"))
        his.append(np.searchsorted(xs[:, 0], ch.max() + f32(0.21), "right"))
    W = int(max(hh - l for l, hh in zip(los, his)))
    W = max(128, -(-W // 128) * 128)
    wc = W // 128
    lo = [min(max(0, l), N - W) for l in los]

    h_b = h_s.astype(MNP)
    wsoc_r = np.ascontiguousarray(
        W_soc.reshape(EMB, G, RNN).transpose(2, 1, 0).reshape(RNN, G * EMB)
    ).astype(MNP)
    wembT = np.ascontiguousarray(W_emb.T)
    b_embsoc = np.ascontiguousarray(
        np.concatenate([np.asarray(b_emb, f32), np.asarray(b_soc, f32)])[:, None])
    wihT = np.ascontiguousarray(W_ih.T)
    whhT = np.ascontiguousarray(W_hh.T)
    bgates_ih = np.ascontiguousarray(np.asarray(b_ih, f32).reshape(4, RNN).T)
    bgates_hh = np.ascontiguousarray(np.asarray(b_hh, f32).reshape(4, RNN).T)
    woutT = np.ascontiguousarray(W_out.T)
    bout = np.ascontiguousarray(np.asarray(b_out, f32)[:, None])
    ramp = _make_ramp()
    actc = np.ascontiguousarray(
        np.broadcast_to(np.array([0.5, 9.0], f32)[None, :], (128, 2)))

    in_maps = []
    for k in range(NCORE):
        sl = slice(k * NC_CHUNK, (k + 1) * NC_CHUNK)
        win = slice(lo[k], lo[k] + W)
        eye_r = np.zeros((128, W), MNP)
        idx = np.arange(128)
        ms = k * NC_CHUNK + idx - lo[k]
        eye_r[ms % 128, (ms // 128) * 128 + idx] = MNP(1000.0)
        xw = xs[win]
        hw = h_b[win]
        in_maps.append({
            "xabs_r": np.ascontiguousarray(
                xw.reshape(wc, 128, 2).transpose(1, 0, 2).reshape(128, 2 * wc)),
            "xnb": np.ascontiguousarray(
                np.broadcast_to(xs[sl, 0][None, :], (128, NC_CHUNK))),
            "ynb": np.ascontiguousarray(
                np.broadcast_to(xs[sl, 1][None, :], (128, NC_CHUNK))),
            "actc": actc,
            "eye_r": eye_r,
            "ramp_in": ramp,
            "h_winp": np.ascontiguousarray(
                hw.reshape(wc, 128, RNN).transpose(1, 0, 2).reshape(128, W)),
            "wsoc_r": wsoc_r,
            "wembT": wembT,
            "xoffT": np.ascontiguousarray(xoff_s[sl].T),
            "b_embsoc": b_embsoc,
            "wihT": wihT,
            "whhT": whhT,
            "bgates_ih": bgates_ih,
            "bgates_hh": bgates_hh,
            "hT_c": np.ascontiguousarray(h_s[sl].T),
            "cT_c": np.ascontiguousarray(c_s[sl].T),
            "woutT": woutT,
            "bout": bout,
        })
    return in_maps, perm, wc


def kernel(**inputs):
    in_maps, perm, wc = _prep_inputs(**inputs)
    nc = _get_program(wc)
    res = run_bass_kernel_spmd(nc, in_maps, list(range(NCORE)))
    outT = np.concatenate([res.results[k]["outT"] for k in range(NCORE)],
                          axis=1)
    out_sorted = outT.T
    out = np.empty_like(out_sorted)
    out[perm] = out_sorted
    return tuple(np.ascontiguousarray(out[:, i * NMIX:(i + 1) * NMIX])
                 for i in range(6))


# revision 25
# speedup vs baseline: 1.0990x; 1.0990x over previous
"""Social-LSTM single-step kernel for 8 Trainium2 NeuronCores.

Sort pedestrians by x on the host; shard sorted targets across 8 cores
(128 each). Grid neighbors lie within +-0.2 in x, so each core gets a
host-sliced window of W sorted neighbors. On chip, each core computes
per-pair grid-cell codes, expands them to bf16 one-hot masks against a
code ramp (wide is_equal, 2x DVE mode), and accumulates
social^T[h, n] per grid cell in PSUM via TensorE matmuls with the
neighbor hidden states stationary. Social pooling, embedding, LSTM and
the output projection run on-chip; the host only permutes/slices
inputs and inverse-permutes the output shards.
"""
import numpy as np
import ml_dtypes

from concourse import bass, mybir
from concourse.tile import TileContext, ScopedClock
from concourse.bass_utils import run_bass_kernel_spmd

F32 = mybir.dt.float32
I32 = mybir.dt.int32
BF16 = mybir.dt.bfloat16
ALU = mybir.AluOpType
ACT = mybir.ActivationFunctionType
BF = ml_dtypes.bfloat16

N = 1024
RNN = 128
EMB = 64
GS = 8
G = GS * GS
NMIX = 20
NCORE = 8
NC_CHUNK = N // NCORE
MDT = BF16
MNP = BF
RCH = 4                    # ramp/mask column chunks
RC_G = G // RCH            # 16 cells per ramp chunk
RC_W = RC_G * NC_CHUNK     # 2048 mask columns per chunk
PSG = [12, 12, 12, 12, 12, 4]   # psum group sizes (cells)


def _patched_drain(self, tick_clock, wait_clock):
    nop_inst = self.nc.sync.nop()
    wait_clock.add_sem_waits(nop_inst.ins, ScopedClock({None: tick_clock.global_clock}))
    si = nop_inst.ins.sync_info
    waits = list(si.on_wait or [])
    si.on_wait = waits[:1]
    for i in range(1, len(waits)):
        extra = self.nc.sync.nop()
        extra.ins.sync_info = mybir.SyncInfo(on_update=[], on_wait=[waits[i]])
    self.nc.sync.drain()
    self.nc.all_engine_barrier()
    popped = self.nc._tile_sem_poison_stack.pop()
    assert popped is self._sem_poison
    self.nc.clear_and_free_semaphores(list(self.sems.allocated().values()))
    self.nc.all_engine_barrier()


TileContext._drain_and_barrier = _patched_drain


def _split_multi_waits(nc):
    for fn in nc.m.functions:
        for bb in fn.blocks:
            new_insts = []
            for inst in bb.instructions:
                si = getattr(inst, "sync_info", None)
                waits = list(si.on_wait) if si is not None and si.on_wait else []
                if len(waits) > 1:
                    for w in waits[:-1]:
                        new_insts.append(mybir.InstNoOp(
                            name=nc.get_next_instruction_name(), ins=[], outs=[],
                            engine=inst.engine,
                            sync_info=mybir.SyncInfo(on_update=[], on_wait=[w]),
                        ))
                    si.on_wait = [waits[-1]]
                new_insts.append(inst)
            bb.instructions = new_insts


def _build_program(wc):
    W = wc * 128
    nc = bass.Bass(target_bir_lowering=False)

    xabs_r = nc.dram_tensor("xabs_r", [128, 2 * wc], F32, kind="ExternalInput")
    xnb = nc.dram_tensor("xnb", [128, NC_CHUNK], F32, kind="ExternalInput")
    ynb = nc.dram_tensor("ynb", [128, NC_CHUNK], F32, kind="ExternalInput")
    actc = nc.dram_tensor("actc", [128, 2], F32, kind="ExternalInput")
    eye_r = nc.dram_tensor("eye_r", [128, W], MDT, kind="ExternalInput")
    ramp_in = nc.dram_tensor("ramp_in", [RCH * 128, RC_W], MDT, kind="ExternalInput")
    h_winp = nc.dram_tensor("h_winp", [128, W], MDT, kind="ExternalInput")
    wsoc_r = nc.dram_tensor("wsoc_r", [RNN, G * EMB], MDT, kind="ExternalInput")
    wembT = nc.dram_tensor("wembT", [2, EMB], F32, kind="ExternalInput")
    xoffT = nc.dram_tensor("xoffT", [2, NC_CHUNK], F32, kind="ExternalInput")
    b_embsoc = nc.dram_tensor("b_embsoc", [128, 1], F32, kind="ExternalInput")
    wihT = nc.dram_tensor("wihT", [128, 4 * RNN], F32, kind="ExternalInput")
    whhT = nc.dram_tensor("whhT", [RNN, 4 * RNN], F32, kind="ExternalInput")
    bgates_ih = nc.dram_tensor("bgates_ih", [128, 4], F32, kind="ExternalInput")
    bgates_hh = nc.dram_tensor("bgates_hh", [128, 4], F32, kind="ExternalInput")
    hT_c = nc.dram_tensor("hT_c", [RNN, NC_CHUNK], F32, kind="ExternalInput")
    cT_c = nc.dram_tensor("cT_c", [RNN, NC_CHUNK], F32, kind="ExternalInput")
    woutT = nc.dram_tensor("woutT", [RNN, 6 * NMIX], F32, kind="ExternalInput")
    bout = nc.dram_tensor("bout", [6 * NMIX, 1], F32, kind="ExternalInput")
    outT = nc.dram_tensor("outT", [6 * NMIX, NC_CHUNK], F32, kind="ExternalOutput")

    with TileContext(nc) as tc:
        with (
            tc.tile_pool(name="const", bufs=1) as cpool,
            tc.tile_pool(name="masks", bufs=1) as maskpool,
            tc.tile_pool(name="soc", bufs=2) as socpool,
            tc.tile_pool(name="work", bufs=2) as work,
            tc.tile_pool(name="psum", bufs=1, space="PSUM") as pp,
            tc.tile_pool(name="psum_soc", bufs=2, space="PSUM") as pps,
        ):
            # ---- small / latency-critical inputs on the sync queue ----
            xabs_sb = cpool.tile([128, 2 * wc], F32, tag="xabs")
            nc.sync.dma_start(xabs_sb[:, :], xabs_r[:, :])
            xnb_sb = cpool.tile([128, NC_CHUNK], F32, tag="xnb")
            nc.sync.dma_start(xnb_sb[:, :], xnb[:, :])
            ynb_sb = cpool.tile([128, NC_CHUNK], F32, tag="ynb")
            nc.sync.dma_start(ynb_sb[:, :], ynb[:, :])
            actc_sb = cpool.tile([128, 2], F32, tag="actc")
            nc.sync.dma_start(actc_sb[:, :], actc[:, :])
            eye_sb = cpool.tile([128, W], MDT, tag="eye")
            nc.sync.dma_start(eye_sb[:, :], eye_r[:, :])
            xm02 = cpool.tile([128, 2 * wc], F32, tag="xm02")
            nc.vector.tensor_scalar(xm02[:, :], xabs_sb[:, :], 0.2, None,
                                    op0=ALU.add)
            ramp = []
            for c in range(RCH):
                ramp_t = cpool.tile([128, RC_W], MDT, tag=f"ramp{c}")
                nc.sync.dma_start(ramp_t[:, :], ramp_in[c * 128:(c + 1) * 128, :])
                ramp.append(ramp_t)
            h_big = cpool.tile([128, W], MDT, tag="h_big")
            nc.gpsimd.dma_start(h_big[:, :], h_winp[:, :])
            wsoc_sb = cpool.tile([RNN, G * EMB], MDT, tag="wsoc")
            nc.gpsimd.dma_start(wsoc_sb[:, :], wsoc_r[:, :])
            wihT_sb = cpool.tile([128, 4 * RNN], F32, tag="wihT")
            nc.gpsimd.dma_start(wihT_sb[:, :], wihT[:, :])
            whhT_sb = cpool.tile([RNN, 4 * RNN], F32, tag="whhT")
            nc.gpsimd.dma_start(whhT_sb[:, :], whhT[:, :])
            woutT_sb = cpool.tile([RNN, 6 * NMIX], F32, tag="woutT")
            nc.gpsimd.dma_start(woutT_sb[:, :], woutT[:, :])
            wembT_sb = cpool.tile([2, EMB], F32, tag="wembT")
            nc.sync.dma_start(wembT_sb[:, :], wembT[:, :])
            xoffT_sb = cpool.tile([2, NC_CHUNK], F32, tag="xoffT")
            nc.sync.dma_start(xoffT_sb[:, :], xoffT[:, :])
            b_es_sb = cpool.tile([128, 1], F32, tag="b_embsoc")
            nc.sync.dma_start(b_es_sb[:, :], b_embsoc[:, :])
            hT_sb = cpool.tile([RNN, NC_CHUNK], F32, tag="hT")
            nc.sync.dma_start(hT_sb[:, :], hT_c[:, :])
            cT_sb = cpool.tile([RNN, NC_CHUNK], F32, tag="cT")
            nc.sync.dma_start(cT_sb[:, :], cT_c[:, :])
            bgi_sb = cpool.tile([128, 4], F32, tag="bgates_ih")
            nc.sync.dma_start(bgi_sb[:, :], bgates_ih[:, :])
            bgh_sb = cpool.tile([128, 4], F32, tag="bgates_hh")
            nc.sync.dma_start(bgh_sb[:, :], bgates_hh[:, :])
            bout_sb = cpool.tile([6 * NMIX, 1], F32, tag="bout")
            nc.sync.dma_start(bout_sb[:, :], bout[:, :])

            # ---- cell codes per neighbor chunk ----
            # code = 108 - t2x - 11*t2y,  t2 = rint(relu(9 - relu(v + 0.5)))
            # (ACT converts f32->i32 round-to-nearest-even; HW verified)
            cells = []
            for mc in range(wc):
                vx = work.tile([128, NC_CHUNK], F32, tag="vx")
                nc.vector.tensor_scalar(vx[:, :], xnb_sb[:, :],
                                        xm02[:, 2 * mc:2 * mc + 1], -20.0,
                                        op0=ALU.subtract, op1=ALU.mult)
                vy = work.tile([128, NC_CHUNK], F32, tag="vy")
                nc.vector.tensor_scalar(vy[:, :], ynb_sb[:, :],
                                        xm02[:, 2 * mc + 1:2 * mc + 2], -20.0,
                                        op0=ALU.subtract, op1=ALU.mult)
                t2x = work.tile([128, NC_CHUNK], I32, tag="t2x")
                t2y = work.tile([128, NC_CHUNK], I32, tag="t2y")
                t1x = work.tile([128, NC_CHUNK], F32, tag="t1x")
                nc.scalar.activation(t1x[:, :], vx[:, :], ACT.Relu,
                                     bias=actc_sb[:, 0:1], scale=1.0)
                nc.scalar.activation(t2x[:, :], t1x[:, :], ACT.Relu,
                                     bias=actc_sb[:, 1:2], scale=-1.0)
                t1y = work.tile([128, NC_CHUNK], F32, tag="t1y")
                nc.scalar.activation(t1y[:, :], vy[:, :], ACT.Relu,
                                     bias=actc_sb[:, 0:1], scale=1.0)
                nc.scalar.activation(t2y[:, :], t1y[:, :], ACT.Relu,
                                     bias=actc_sb[:, 1:2], scale=-1.0)
                u = work.tile([128, NC_CHUNK], I32, tag="u")
                nc.vector.tensor_scalar(u[:, :], t2y[:, :], -11, 108,
                                        op0=ALU.mult, op1=ALU.add)
                cc = work.tile([128, NC_CHUNK], MDT, tag=f"cell{mc}")
                nc.vector.tensor_tensor(cc[:, :], u[:, :], t2x[:, :],
                                        op=ALU.subtract)
                nc.vector.tensor_tensor(cc[:, :], cc[:, :],
                                        eye_sb[:, mc * 128:(mc + 1) * 128],
                                        op=ALU.add)
                cells.append(cc)

            # ---- masks: chunk-major so PE group g can start early ----
            masks = {}
            for c in range(RCH):
                for mc in range(wc):
                    m = maskpool.tile([128, RC_W], MDT, tag=f"m{mc}g{c}")
                    cb = cells[mc][:, :].unsqueeze(1).broadcast_to(
                        [128, RC_G, NC_CHUNK])
                    nc.vector.tensor_tensor(m[:, :], cb, ramp[c][:, :],
                                            op=ALU.is_equal)
                    masks[(mc, c)] = m

            # ---- social matmuls + pooling, double-buffered psum groups ----
            xin_ps = pp.tile([128, NC_CHUNK], F32, tag="xin_ps")
            g0 = 0
            for gi, gsz in enumerate(PSG):
                soc_ps = pps.tile([128, 12 * NC_CHUNK], F32, tag="soc_ps")
                for mc in range(wc):
                    for s in range(gsz // 4):
                        cell0 = g0 + s * 4
                        c = cell0 // RC_G
                        off = (cell0 % RC_G) * NC_CHUNK
                        nc.tensor.matmul(
                            soc_ps[:, s * 512:(s + 1) * 512],
                            h_big[:, mc * 128:(mc + 1) * 128],
                            masks[(mc, c)][:, off:off + 512],
                            start=(mc == 0), stop=(mc == wc - 1))
                soc_sb = socpool.tile([128, 12 * NC_CHUNK], MDT, tag="soc_sb")
                nc.scalar.activation(soc_sb[:, :gsz * NC_CHUNK],
                                     soc_ps[:, :gsz * NC_CHUNK], ACT.Copy,
                                     bias=0.0, scale=1.0)
                for gl in range(gsz):
                    g = g0 + gl
                    nc.tensor.matmul(xin_ps[EMB:, :],
                                     wsoc_sb[:, g * EMB:(g + 1) * EMB],
                                     soc_sb[:, gl * NC_CHUNK:(gl + 1) * NC_CHUNK],
                                     start=(g == 0), stop=(g == G - 1))
                g0 += gsz

            # ---- embedding ----
            nc.tensor.matmul(xin_ps[:EMB, :], wembT_sb[:, :], xoffT_sb[:, :],
                             start=True, stop=True)
            xinT = work.tile([128, NC_CHUNK], F32, tag="xinT")
            nc.scalar.activation(xinT[:, :], xin_ps[:, :], ACT.Relu,
                                 bias=b_es_sb[:, 0:1], scale=1.0)

            # ---- LSTM ----
            bg_sb = cpool.tile([128, 4], F32, tag="bgates")
            nc.vector.tensor_tensor(bg_sb[:, :], bgi_sb[:, :], bgh_sb[:, :],
                                    op=ALU.add)
            acts = []
            for q in range(4):
                g_ps = pp.tile([128, NC_CHUNK], F32, tag="g_ps")
                nc.tensor.matmul(g_ps[:, :], wihT_sb[:, q * RNN:(q + 1) * RNN],
                                 xinT[:, :], start=True, stop=False)
                nc.tensor.matmul(g_ps[:, :], whhT_sb[:, q * RNN:(q + 1) * RNN],
                                 hT_sb[:, :], start=False, stop=True)
                gq = work.tile([128, NC_CHUNK], F32, tag=f"gate{q}")
                func = ACT.Tanh if q == 2 else ACT.Sigmoid
                nc.scalar.activation(gq[:, :], g_ps[:, :], func,
                                     bias=bg_sb[:, q:q + 1], scale=1.0)
                acts.append(gq)

            fc = work.tile([128, NC_CHUNK], F32, tag="fc")
            nc.vector.tensor_tensor(fc[:, :], acts[1][:, :], cT_sb[:, :],
                                    op=ALU.mult)
            ig = work.tile([128, NC_CHUNK], F32, tag="ig")
            nc.vector.tensor_tensor(ig[:, :], acts[0][:, :], acts[2][:, :],
                                    op=ALU.mult)
            cnew = work.tile([128, NC_CHUNK], F32, tag="cnew")
            nc.vector.tensor_tensor(cnew[:, :], fc[:, :], ig[:, :], op=ALU.add)
            tc_t = work.tile([128, NC_CHUNK], F32, tag="tc")
            nc.scalar.activation(tc_t[:, :], cnew[:, :], ACT.Tanh,
                                 bias=0.0, scale=1.0)
            hn = work.tile([128, NC_CHUNK], F32, tag="hn")
            nc.vector.tensor_tensor(hn[:, :], acts[3][:, :], tc_t[:, :],
                                    op=ALU.mult)

            # ---- output projection ----
            out_ps = pp.tile([6 * NMIX, NC_CHUNK], F32, tag="g_ps")
            nc.tensor.matmul(out_ps[:, :], woutT_sb[:, :], hn[:, :],
                             start=True, stop=True)
            outT_sb = work.tile([6 * NMIX, NC_CHUNK], F32, tag="outT")
            nc.vector.tensor_scalar(outT_sb[:, :], out_ps[:, :],
                                    bout_sb[:, 0:1], None, op0=ALU.add)
            nc.sync.dma_start(outT[:, :], outT_sb[:, :])

    _split_multi_waits(nc)
    return nc


_NC_CACHE = {}


def _get_program(wc):
    if wc not in _NC_CACHE:
        _NC_CACHE[wc] = _build_program(wc)
    return _NC_CACHE[wc]


def _make_ramp():
    gy, gx, n = np.meshgrid(np.arange(GS), np.arange(GS), np.arange(NC_CHUNK),
                            indexing="ij")
    vals = (12 + gx + 11 * gy).reshape(1, G * NC_CHUNK)
    full = np.broadcast_to(vals, (128, G * NC_CHUNK)).astype(MNP)
    # chunked layout: [RCH*128, RC_W], chunk c = rows 128c..128c+127
    return np.ascontiguousarray(
        full.reshape(128, RCH, RC_W).transpose(1, 0, 2).reshape(RCH * 128, RC_W))


def _prep_inputs(xoff, xabs, h0, c0, W_emb, b_emb, W_soc, b_soc,
                 W_ih, W_hh, b_ih, b_hh, W_out, b_out):
    f32 = np.float32
    xoff = np.asarray(xoff, f32)
    xabs = np.asarray(xabs, f32)
    h = np.asarray(h0, f32)[0]
    c = np.asarray(c0, f32)[0]
    W_emb = np.asarray(W_emb, f32)
    W_soc = np.asarray(W_soc, f32)
    W_ih = np.asarray(W_ih, f32)
    W_hh = np.asarray(W_hh, f32)
    W_out = np.asarray(W_out, f32)

    perm = np.argsort(xabs[:, 0], kind="stable")
    xs = xabs[perm]
    xoff_s = xoff[perm]
    h_s = h[perm]
    c_s = c[perm]

    los, his = [], []
    for k in range(NCORE):
        ch = xs[k * NC_CHUNK:(k + 1) * NC_CHUNK, 0]
        los.append(np.searchsorted(xs[:, 0], ch.min() - f32(0.21), "left"))
        his.append(np.searchsorted(xs[:, 0], ch.max() + f32(0.21), "right"))
    W = int(max(hh - l for l, hh in zip(los, his)))
    W = max(128, -(-W // 128) * 128)
    wc = W // 128
    lo = [min(max(0, l), N - W) for l in los]

    h_b = h_s.astype(MNP)
    wsoc_r = np.ascontiguousarray(
        W_soc.reshape(EMB, G, RNN).transpose(2, 1, 0).reshape(RNN, G * EMB)
    ).astype(MNP)
    wembT = np.ascontiguousarray(W_emb.T)
    b_embsoc = np.ascontiguousarray(
        np.concatenate([np.asarray(b_emb, f32), np.asarray(b_soc, f32)])[:, None])
    wihT = np.ascontiguousarray(W_ih.T)
    whhT = np.ascontiguousarray(W_hh.T)
    bgates_ih = np.ascontiguousarray(np.asarray(b_ih, f32).reshape(4, RNN).T)
    bgates_hh = np.ascontiguousarray(np.asarray(b_hh, f32).reshape(4, RNN).T)
    woutT = np.ascontiguousarray(W_out.T)
    bout = np.ascontiguousarray(np.asarray(b_out, f32)[:, None])
    ramp = _make_ramp()
    actc = np.ascontiguousarray(
        np.broadcast_to(np.array([0.5, 9.0], f32)[None, :], (128, 2)))

    in_maps = []
    for k in range(NCORE):
        sl = slice(k * NC_CHUNK, (k + 1) * NC_CHUNK)
        win = slice(lo[k], lo[k] + W)
        eye_r = np.zeros((128, W), MNP)
        idx = np.arange(128)
        ms = k * NC_CHUNK + idx - lo[k]
        eye_r[ms % 128, (ms // 128) * 128 + idx] = MNP(1000.0)
        xw = xs[win]
        hw = h_b[win]
        in_maps.append({
            "xabs_r": np.ascontiguousarray(
                xw.reshape(wc, 128, 2).transpose(1, 0, 2).reshape(128, 2 * wc)),
            "xnb": np.ascontiguousarray(
                np.broadcast_to(xs[sl, 0][None, :], (128, NC_CHUNK))),
            "ynb": np.ascontiguousarray(
                np.broadcast_to(xs[sl, 1][None, :], (128, NC_CHUNK))),
            "actc": actc,
            "eye_r": eye_r,
            "ramp_in": ramp,
            "h_winp": np.ascontiguousarray(
                hw.reshape(wc, 128, RNN).transpose(1, 0, 2).reshape(128, W)),
            "wsoc_r": wsoc_r,
            "wembT": wembT,
            "xoffT": np.ascontiguousarray(xoff_s[sl].T),
            "b_embsoc": b_embsoc,
            "wihT": wihT,
            "whhT": whhT,
            "bgates_ih": bgates_ih,
            "bgates_hh": bgates_hh,
            "hT_c": np.ascontiguousarray(h_s[sl].T),
            "cT_c": np.ascontiguousarray(c_s[sl].T),
            "woutT": woutT,
            "bout": bout,
        })
    return in_maps, perm, wc


def kernel(**inputs):
    in_maps, perm, wc = _prep_inputs(**inputs)
    nc = _get_program(wc)
    res = run_bass_kernel_spmd(nc, in_maps, list(range(NCORE)))
    outT = np.concatenate([res.results[k]["outT"] for k in range(NCORE)],
                          axis=1)
    out_sorted = outT.T
    out = np.empty_like(out_sorted)
    out[perm] = out_sorted
    return tuple(np.ascontiguousarray(out[:, i * NMIX:(i + 1) * NMIX])
                 for i in range(6))


# revision 37
# speedup vs baseline: 16014.6054x; 14572.1704x over previous
"""Social-LSTM single-step kernel for 8 Trainium2 NeuronCores.

Sort pedestrians by x on the host; shard sorted targets across 8 cores
(128 each). Grid neighbors lie within +-0.2 in x, so each core gets a
host-sliced window of W sorted neighbors. On chip, each core computes
per-pair grid-cell codes, expands them to bf16 one-hot masks against a
code ramp (wide is_equal, 2x DVE mode), and accumulates
social^T[h, n] per grid cell in PSUM via TensorE matmuls with the
neighbor hidden states stationary. Social pooling, embedding, LSTM and
the output projection run on-chip; the host only permutes/slices
inputs and inverse-permutes the output shards.
"""
import numpy as np
import ml_dtypes

from concourse import bass, mybir
from concourse.tile import TileContext, ScopedClock
from concourse.bass_utils import run_bass_kernel_spmd

F32 = mybir.dt.float32
I32 = mybir.dt.int32
BF16 = mybir.dt.bfloat16
ALU = mybir.AluOpType
ACT = mybir.ActivationFunctionType
BF = ml_dtypes.bfloat16

N = 1024
RNN = 128
EMB = 64
GS = 8
G = GS * GS
NMIX = 20
NCORE = 8
NC_CHUNK = N // NCORE
MDT = BF16
MNP = BF
RCH = 4                    # ramp/mask column chunks
RC_G = G // RCH            # 16 cells per ramp chunk
RC_W = RC_G * NC_CHUNK     # 2048 mask columns per chunk
PSG = [12, 12, 12, 12, 12, 4]   # psum group sizes (cells)


def _patched_drain(self, tick_clock, wait_clock):
    nop_inst = self.nc.sync.nop()
    wait_clock.add_sem_waits(nop_inst.ins, ScopedClock({None: tick_clock.global_clock}))
    si = nop_inst.ins.sync_info
    waits = list(si.on_wait or [])
    si.on_wait = waits[:1]
    for i in range(1, len(waits)):
        extra = self.nc.sync.nop()
        extra.ins.sync_info = mybir.SyncInfo(on_update=[], on_wait=[waits[i]])
    self.nc.sync.drain()
    popped = self.nc._tile_sem_poison_stack.pop()
    assert popped is self._sem_poison
    # Bass's preamble re-clears all kernel sems at the start of the next
    # execution, so exit-time clear instructions are redundant.
    sems = list(self.sems.allocated().values())
    sem_nums = [s.num for s in sems]
    self.nc._state.prepend_free_semaphores(sem_nums)
    for poison_set in self.nc._tile_sem_poison_stack:
        poison_set.update(sem_nums)


TileContext._drain_and_barrier = _patched_drain


def _split_multi_waits(nc):
    for fn in nc.m.functions:
        for bb in fn.blocks:
            new_insts = []
            for inst in bb.instructions:
                si = getattr(inst, "sync_info", None)
                waits = list(si.on_wait) if si is not None and si.on_wait else []
                if len(waits) > 1:
                    for w in waits[:-1]:
                        new_insts.append(mybir.InstNoOp(
                            name=nc.get_next_instruction_name(), ins=[], outs=[],
                            engine=inst.engine,
                            sync_info=mybir.SyncInfo(on_update=[], on_wait=[w]),
                        ))
                    si.on_wait = [waits[-1]]
                new_insts.append(inst)
            bb.instructions = new_insts


def _build_program(wc):
    W = wc * 128
    nc = bass.Bass(target_bir_lowering=False)

    xabs_r = nc.dram_tensor("xabs_r", [128, 2 * wc], F32, kind="ExternalInput")
    xnb = nc.dram_tensor("xnb", [128, NC_CHUNK], F32, kind="ExternalInput")
    ynb = nc.dram_tensor("ynb", [128, NC_CHUNK], F32, kind="ExternalInput")
    actc = nc.dram_tensor("actc", [128, 2], F32, kind="ExternalInput")
    eye_r = nc.dram_tensor("eye_r", [128, W], MDT, kind="ExternalInput")
    ramp_in = nc.dram_tensor("ramp_in", [RCH * 128, RC_W], MDT, kind="ExternalInput")
    h_winp = nc.dram_tensor("h_winp", [128, W], MDT, kind="ExternalInput")
    wsoc_r = nc.dram_tensor("wsoc_r", [RNN, G * EMB], MDT, kind="ExternalInput")
    wembT = nc.dram_tensor("wembT", [2, EMB], F32, kind="ExternalInput")
    xoffT = nc.dram_tensor("xoffT", [2, NC_CHUNK], F32, kind="ExternalInput")
    b_embsoc = nc.dram_tensor("b_embsoc", [128, 1], F32, kind="ExternalInput")
    wihT = nc.dram_tensor("wihT", [128, 4 * RNN], F32, kind="ExternalInput")
    whhT = nc.dram_tensor("whhT", [RNN, 4 * RNN], F32, kind="ExternalInput")
    bgates_ih = nc.dram_tensor("bgates_ih", [128, 4], F32, kind="ExternalInput")
    bgates_hh = nc.dram_tensor("bgates_hh", [128, 4], F32, kind="ExternalInput")
    hT_c = nc.dram_tensor("hT_c", [RNN, NC_CHUNK], F32, kind="ExternalInput")
    cT_c = nc.dram_tensor("cT_c", [RNN, NC_CHUNK], F32, kind="ExternalInput")
    woutT = nc.dram_tensor("woutT", [RNN, 6 * NMIX], F32, kind="ExternalInput")
    bout = nc.dram_tensor("bout", [6 * NMIX, 1], F32, kind="ExternalInput")
    outT = nc.dram_tensor("outT", [6 * NMIX, NC_CHUNK], F32, kind="ExternalOutput")

    with TileContext(nc) as tc:
        with (
            tc.tile_pool(name="const", bufs=1) as cpool,
            tc.tile_pool(name="masks", bufs=1) as maskpool,
            tc.tile_pool(name="soc", bufs=2) as socpool,
            tc.tile_pool(name="work", bufs=2) as work,
            tc.tile_pool(name="psum", bufs=1, space="PSUM") as pp,
            tc.tile_pool(name="psum_soc", bufs=2, space="PSUM") as pps,
        ):
            # ---- small / latency-critical inputs on the sync queue ----
            xabs_sb = cpool.tile([128, 2 * wc], F32, tag="xabs")
            nc.sync.dma_start(xabs_sb[:, :], xabs_r[:, :])
            xnb_sb = cpool.tile([128, NC_CHUNK], F32, tag="xnb")
            nc.sync.dma_start(xnb_sb[:, :], xnb[:, :])
            ynb_sb = cpool.tile([128, NC_CHUNK], F32, tag="ynb")
            nc.sync.dma_start(ynb_sb[:, :], ynb[:, :])
            actc_sb = cpool.tile([128, 2], F32, tag="actc")
            nc.sync.dma_start(actc_sb[:, :], actc[:, :])
            eye_sb = cpool.tile([128, W], MDT, tag="eye")
            nc.sync.dma_start(eye_sb[:, :], eye_r[:, :])
            xm02 = cpool.tile([128, 2 * wc], F32, tag="xm02")
            nc.vector.tensor_scalar(xm02[:, :], xabs_sb[:, :], 0.2, None,
                                    op0=ALU.add)
            # allocation order fixed (affects SBUF offsets / DVE port
            # behavior); DMA issue order by need-by time.
            h_big = cpool.tile([128, W], MDT, tag="h_big")
            ramp = []
            for c in range(RCH):
                ramp_t = cpool.tile([128, RC_W], MDT, tag=f"ramp{c}")
                ramp.append(ramp_t)
            nc.sync.dma_start(ramp[0][:, :], ramp_in[0:128, :])
            nc.sync.dma_start(h_big[:, :], h_winp[:, :])
            for c in range(1, RCH):
                nc.sync.dma_start(ramp[c][:, :], ramp_in[c * 128:(c + 1) * 128, :])
            wsoc_sb = cpool.tile([RNN, G * EMB], MDT, tag="wsoc")
            nc.scalar.dma_start(wsoc_sb[:, :], wsoc_r[:, :])
            wihT_sb = cpool.tile([128, 4 * RNN], F32, tag="wihT")
            nc.scalar.dma_start(wihT_sb[:, :], wihT[:, :])
            whhT_sb = cpool.tile([RNN, 4 * RNN], F32, tag="whhT")
            nc.scalar.dma_start(whhT_sb[:, :], whhT[:, :])
            woutT_sb = cpool.tile([RNN, 6 * NMIX], F32, tag="woutT")
            nc.scalar.dma_start(woutT_sb[:, :], woutT[:, :])
            wembT_sb = cpool.tile([2, EMB], F32, tag="wembT")
            nc.sync.dma_start(wembT_sb[:, :], wembT[:, :])
            xoffT_sb = cpool.tile([2, NC_CHUNK], F32, tag="xoffT")
            nc.sync.dma_start(xoffT_sb[:, :], xoffT[:, :])
            b_es_sb = cpool.tile([128, 1], F32, tag="b_embsoc")
            nc.sync.dma_start(b_es_sb[:, :], b_embsoc[:, :])
            hT_sb = cpool.tile([RNN, NC_CHUNK], F32, tag="hT")
            nc.sync.dma_start(hT_sb[:, :], hT_c[:, :])
            cT_sb = cpool.tile([RNN, NC_CHUNK], F32, tag="cT")
            nc.sync.dma_start(cT_sb[:, :], cT_c[:, :])
            bgi_sb = cpool.tile([128, 4], F32, tag="bgates_ih")
            nc.sync.dma_start(bgi_sb[:, :], bgates_ih[:, :])
            bgh_sb = cpool.tile([128, 4], F32, tag="bgates_hh")
            nc.sync.dma_start(bgh_sb[:, :], bgates_hh[:, :])
            bout_sb = cpool.tile([6 * NMIX, 1], F32, tag="bout")
            nc.sync.dma_start(bout_sb[:, :], bout[:, :])

            # ---- cell codes per neighbor chunk ----
            # code = 108 - t2x - 11*t2y,  t2 = rint(relu(9 - relu(v + 0.5)))
            # (ACT converts f32->i32 round-to-nearest-even; HW verified)
            cells = []
            for mc in range(wc):
                vx = work.tile([128, NC_CHUNK], F32, tag="vx")
                nc.vector.tensor_scalar(vx[:, :], xnb_sb[:, :],
                                        xm02[:, 2 * mc:2 * mc + 1], -20.0,
                                        op0=ALU.subtract, op1=ALU.mult)
                vy = work.tile([128, NC_CHUNK], F32, tag="vy")
                nc.vector.tensor_scalar(vy[:, :], ynb_sb[:, :],
                                        xm02[:, 2 * mc + 1:2 * mc + 2], -20.0,
                                        op0=ALU.subtract, op1=ALU.mult)
                t2x = work.tile([128, NC_CHUNK], I32, tag="t2x")
                t2y = work.tile([128, NC_CHUNK], I32, tag="t2y")
                if mc == 0:
                    # DVE-only clamp chain for the first chunk: DVE is idle
                    # this early while ACT waits for its table load.
                    for v, t2 in ((vx, t2x), (vy, t2y)):
                        t1 = work.tile([128, NC_CHUNK], F32, tag="t1d")
                        nc.vector.tensor_scalar(t1[:, :], v[:, :], 0.5, 0.0,
                                                op0=ALU.add, op1=ALU.max)
                        t9 = work.tile([128, NC_CHUNK], F32, tag="t9d")
                        nc.vector.tensor_scalar(t9[:, :], t1[:, :], -1.0, 9.0,
                                                op0=ALU.mult, op1=ALU.add)
                        nc.vector.tensor_scalar(t2[:, :], t9[:, :], 0.0, None,
                                                op0=ALU.max)
                else:
                    t1x = work.tile([128, NC_CHUNK], F32, tag="t1x")
                    nc.scalar.activation(t1x[:, :], vx[:, :], ACT.Relu,
                                         bias=actc_sb[:, 0:1], scale=1.0)
                    nc.scalar.activation(t2x[:, :], t1x[:, :], ACT.Relu,
                                         bias=actc_sb[:, 1:2], scale=-1.0)
                    t1y = work.tile([128, NC_CHUNK], F32, tag="t1y")
                    nc.scalar.activation(t1y[:, :], vy[:, :], ACT.Relu,
                                         bias=actc_sb[:, 0:1], scale=1.0)
                    nc.scalar.activation(t2y[:, :], t1y[:, :], ACT.Relu,
                                         bias=actc_sb[:, 1:2], scale=-1.0)
                u = work.tile([128, NC_CHUNK], I32, tag="u")
                nc.vector.tensor_scalar(u[:, :], t2y[:, :], -11, 108,
                                        op0=ALU.mult, op1=ALU.add)
                cc = work.tile([128, NC_CHUNK], MDT, tag=f"cell{mc}")
                nc.vector.tensor_tensor(cc[:, :], u[:, :], t2x[:, :],
                                        op=ALU.subtract)
                nc.vector.tensor_tensor(cc[:, :], cc[:, :],
                                        eye_sb[:, mc * 128:(mc + 1) * 128],
                                        op=ALU.add)
                cells.append(cc)

            # ---- masks: chunk-major so PE group g can start early ----
            masks = {}
            for c in range(RCH):
                for mc in range(wc):
                    m = maskpool.tile([128, RC_W], MDT, tag=f"m{mc}g{c}")
                    cb = cells[mc][:, :].unsqueeze(1).broadcast_to(
                        [128, RC_G, NC_CHUNK])
                    nc.vector.tensor_tensor(m[:, :], cb, ramp[c][:, :],
                                            op=ALU.is_equal)
                    masks[(mc, c)] = m

            # ---- social matmuls + pooling, double-buffered psum groups ----
            xin_ps = pp.tile([128, NC_CHUNK], F32, tag="xin_ps")
            g0 = 0
            for gi, gsz in enumerate(PSG):
                soc_ps = pps.tile([128, 12 * NC_CHUNK], F32, tag="soc_ps")
                for mc in range(wc):
                    for s in range(gsz // 4):
                        cell0 = g0 + s * 4
                        c = cell0 // RC_G
                        off = (cell0 % RC_G) * NC_CHUNK
                        nc.tensor.matmul(
                            soc_ps[:, s * 512:(s + 1) * 512],
                            h_big[:, mc * 128:(mc + 1) * 128],
                            masks[(mc, c)][:, off:off + 512],
                            start=(mc == 0), stop=(mc == wc - 1))
                soc_sb = socpool.tile([128, 12 * NC_CHUNK], MDT, tag="soc_sb")
                nc.scalar.activation(soc_sb[:, :gsz * NC_CHUNK],
                                     soc_ps[:, :gsz * NC_CHUNK], ACT.Copy,
                                     bias=0.0, scale=1.0)
                for gl in range(gsz):
                    g = g0 + gl
                    nc.tensor.matmul(xin_ps[EMB:, :],
                                     wsoc_sb[:, g * EMB:(g + 1) * EMB],
                                     soc_sb[:, gl * NC_CHUNK:(gl + 1) * NC_CHUNK],
                                     start=(g == 0), stop=(g == G - 1))
                g0 += gsz

            # ---- embedding ----
            nc.tensor.matmul(xin_ps[:EMB, :], wembT_sb[:, :], xoffT_sb[:, :],
                             start=True, stop=True)
            xinT = work.tile([128, NC_CHUNK], F32, tag="xinT")
            nc.scalar.activation(xinT[:, :], xin_ps[:, :], ACT.Relu,
                                 bias=b_es_sb[:, 0:1], scale=1.0)

            # ---- LSTM ----
            bg_sb = cpool.tile([128, 4], F32, tag="bgates")
            nc.vector.tensor_tensor(bg_sb[:, :], bgi_sb[:, :], bgh_sb[:, :],
                                    op=ALU.add)
            acts = []
            for q in range(4):
                g_ps = pp.tile([128, NC_CHUNK], F32, tag="g_ps")
                nc.tensor.matmul(g_ps[:, :], wihT_sb[:, q * RNN:(q + 1) * RNN],
                                 xinT[:, :], start=True, stop=False)
                nc.tensor.matmul(g_ps[:, :], whhT_sb[:, q * RNN:(q + 1) * RNN],
                                 hT_sb[:, :], start=False, stop=True)
                gq = work.tile([128, NC_CHUNK], F32, tag=f"gate{q}")
                func = ACT.Tanh if q == 2 else ACT.Sigmoid
                nc.scalar.activation(gq[:, :], g_ps[:, :], func,
                                     bias=bg_sb[:, q:q + 1], scale=1.0)
                acts.append(gq)

            fc = work.tile([128, NC_CHUNK], F32, tag="fc")
            nc.vector.tensor_tensor(fc[:, :], acts[1][:, :], cT_sb[:, :],
                                    op=ALU.mult)
            ig = work.tile([128, NC_CHUNK], F32, tag="ig")
            nc.vector.tensor_tensor(ig[:, :], acts[0][:, :], acts[2][:, :],
                                    op=ALU.mult)
            cnew = work.tile([128, NC_CHUNK], F32, tag="cnew")
            nc.vector.tensor_tensor(cnew[:, :], fc[:, :], ig[:, :], op=ALU.add)
            tc_t = work.tile([128, NC_CHUNK], F32, tag="tc")
            nc.scalar.activation(tc_t[:, :], cnew[:, :], ACT.Tanh,
                                 bias=0.0, scale=1.0)
            hn = work.tile([128, NC_CHUNK], F32, tag="hn")
            nc.vector.tensor_tensor(hn[:, :], acts[3][:, :], tc_t[:, :],
                                    op=ALU.mult)

            # ---- output projection ----
            out_ps = pp.tile([6 * NMIX, NC_CHUNK], F32, tag="g_ps")
            nc.tensor.matmul(out_ps[:, :], woutT_sb[:, :], hn[:, :],
                             start=True, stop=True)
            outT_sb = work.tile([6 * NMIX, NC_CHUNK], F32, tag="outT")
            nc.vector.tensor_scalar(outT_sb[:, :], out_ps[:, :],
                                    bout_sb[:, 0:1], None, op0=ALU.add)
            nc.sync.dma_start(outT[:, :], outT_sb[:, :])

    _split_multi_waits(nc)
    return nc


_NC_CACHE = {}


def _get_program(wc):
    if wc not in _NC_CACHE:
        _NC_CACHE[wc] = _build_program(wc)
    return _NC_CACHE[wc]


def _make_ramp():
    gy, gx, n = np.meshgrid(np.arange(GS), np.arange(GS), np.arange(NC_CHUNK),
                            indexing="ij")
    vals = (12 + gx + 11 * gy).reshape(1, G * NC_CHUNK)
    full = np.broadcast_to(vals, (128, G * NC_CHUNK)).astype(MNP)
    # chunked layout: [RCH*128, RC_W], chunk c = rows 128c..128c+127
    return np.ascontiguousarray(
        full.reshape(128, RCH, RC_W).transpose(1, 0, 2).reshape(RCH * 128, RC_W))


def _prep_inputs(xoff, xabs, h0, c0, W_emb, b_emb, W_soc, b_soc,
                 W_ih, W_hh, b_ih, b_hh, W_out, b_out):
    f32 = np.float32
    xoff = np.asarray(xoff, f32)
    xabs = np.asarray(xabs, f32)
    h = np.asarray(h0, f32)[0]
    c = np.asarray(c0, f32)[0]
    W_emb = np.asarray(W_emb, f32)
    W_soc = np.asarray(W_soc, f32)
    W_ih = np.asarray(W_ih, f32)
    W_hh = np.asarray(W_hh, f32)
    W_out = np.asarray(W_out, f32)

    perm = np.argsort(xabs[:, 0], kind="stable")
    xs = xabs[perm]
    xoff_s = xoff[perm]
    h_s = h[perm]
    c_s = c[perm]

    los, his = [], []
    for k in range(NCORE):
        ch = xs[k * NC_CHUNK:(k + 1) * NC_CHUNK, 0]
        los.append(np.searchsorted(xs[:, 0], ch.min() - f32(0.21), "left"))
        his.append(np.searchsorted(xs[:, 0], ch.max() + f32(0.21), "right"))
    W = int(max(hh - l for l, hh in zip(los, his)))
    W = max(128, -(-W // 128) * 128)
    wc = W // 128
    lo = [min(max(0, l), N - W) for l in los]

    h_b = h_s.astype(MNP)
    wsoc_r = np.ascontiguousarray(
        W_soc.reshape(EMB, G, RNN).transpose(2, 1, 0).reshape(RNN, G * EMB)
    ).astype(MNP)
    wembT = np.ascontiguousarray(W_emb.T)
    b_embsoc = np.ascontiguousarray(
        np.concatenate([np.asarray(b_emb, f32), np.asarray(b_soc, f32)])[:, None])
    wihT = np.ascontiguousarray(W_ih.T)
    whhT = np.ascontiguousarray(W_hh.T)
    bgates_ih = np.ascontiguousarray(np.asarray(b_ih, f32).reshape(4, RNN).T)
    bgates_hh = np.ascontiguousarray(np.asarray(b_hh, f32).reshape(4, RNN).T)
    woutT = np.ascontiguousarray(W_out.T)
    bout = np.ascontiguousarray(np.asarray(b_out, f32)[:, None])
    ramp = _make_ramp()
    actc = np.ascontiguousarray(
        np.broadcast_to(np.array([0.5, 9.0], f32)[None, :], (128, 2)))

    in_maps = []
    for k in range(NCORE):
        sl = slice(k * NC_CHUNK, (k + 1) * NC_CHUNK)
        win = slice(lo[k], lo[k] + W)
        eye_r = np.zeros((128, W), MNP)
        idx = np.arange(128)
        ms = k * NC_CHUNK + idx - lo[k]
        eye_r[ms % 128, (ms // 128) * 128 + idx] = MNP(1000.0)
        xw = xs[win]
        hw = h_b[win]
        in_maps.append({
            "xabs_r": np.ascontiguousarray(
                xw.reshape(wc, 128, 2).transpose(1, 0, 2).reshape(128, 2 * wc)),
            "xnb": np.ascontiguousarray(
                np.broadcast_to(xs[sl, 0][None, :], (128, NC_CHUNK))),
            "ynb": np.ascontiguousarray(
                np.broadcast_to(xs[sl, 1][None, :], (128, NC_CHUNK))),
            "actc": actc,
            "eye_r": eye_r,
            "ramp_in": ramp,
            "h_winp": np.ascontiguousarray(
                hw.reshape(wc, 128, RNN).transpose(1, 0, 2).reshape(128, W)),
            "wsoc_r": wsoc_r,
            "wembT": wembT,
            "xoffT": np.ascontiguousarray(xoff_s[sl].T),
            "b_embsoc": b_embsoc,
            "wihT": wihT,
            "whhT": whhT,
            "bgates_ih": bgates_ih,
            "bgates_hh": bgates_hh,
            "hT_c": np.ascontiguousarray(h_s[sl].T),
            "cT_c": np.ascontiguousarray(c_s[sl].T),
            "woutT": woutT,
            "bout": bout,
        })
    return in_maps, perm, wc


def kernel(**inputs):
    in_maps, perm, wc = _prep_inputs(**inputs)
    nc = _get_program(wc)
    res = run_bass_kernel_spmd(nc, in_maps, list(range(NCORE)))
    outT = np.concatenate([res.results[k]["outT"] for k in range(NCORE)],
                          axis=1)
    out_sorted = outT.T
    out = np.empty_like(out_sorted)
    out[perm] = out_sorted
    return tuple(np.ascontiguousarray(out[:, i * NMIX:(i + 1) * NMIX])
                 for i in range(6))


# revision 40
# speedup vs baseline: 16651.8053x; 1.0398x over previous
"""Social-LSTM single-step kernel for 8 Trainium2 NeuronCores.

Sort pedestrians by x on the host; shard sorted targets across 8 cores
(128 each). Grid neighbors lie within +-0.2 in x, so each core gets a
host-sliced window of W sorted neighbors. On chip, each core computes
per-pair grid-cell codes, expands them to bf16 one-hot masks against a
code ramp (wide is_equal, 2x DVE mode), and accumulates
social^T[h, n] per grid cell in PSUM via TensorE matmuls with the
neighbor hidden states stationary. Social pooling, embedding, LSTM and
the output projection run on-chip; the host only permutes/slices
inputs and inverse-permutes the output shards.
"""
import numpy as np
import ml_dtypes

from concourse import bass, mybir
from concourse.tile import TileContext, ScopedClock
from concourse.bass_utils import run_bass_kernel_spmd

F32 = mybir.dt.float32
I32 = mybir.dt.int32
BF16 = mybir.dt.bfloat16
ALU = mybir.AluOpType
ACT = mybir.ActivationFunctionType
BF = ml_dtypes.bfloat16

N = 1024
RNN = 128
EMB = 64
GS = 8
G = GS * GS
NMIX = 20
NCORE = 8
NC_CHUNK = N // NCORE
MDT = BF16
MNP = BF
RCH = 4                    # ramp/mask column chunks
RC_G = G // RCH            # 16 cells per ramp chunk
RC_W = RC_G * NC_CHUNK     # 2048 mask columns per chunk
PSG = [12, 12, 12, 12, 12, 4]   # psum group sizes (cells)


def _patched_drain(self, tick_clock, wait_clock):
    # The output DMA is enqueued on SP before this drain, so draining SP's
    # queue covers it; every other engine's final work feeds the output
    # transitively and each engine halts at its own stream end. The full
    # global-clock wait list + barrier Tile normally emits is redundant
    # for this kernel (re-execution correctness verified on HW).
    self.nc.sync.drain()
    popped = self.nc._tile_sem_poison_stack.pop()
    assert popped is self._sem_poison
    # Bass's preamble re-clears all kernel sems at the start of the next
    # execution, so exit-time clear instructions are redundant.
    sems = list(self.sems.allocated().values())
    sem_nums = [s.num for s in sems]
    self.nc._state.prepend_free_semaphores(sem_nums)
    for poison_set in self.nc._tile_sem_poison_stack:
        poison_set.update(sem_nums)


TileContext._drain_and_barrier = _patched_drain


def _split_multi_waits(nc):
    for fn in nc.m.functions:
        for bb in fn.blocks:
            new_insts = []
            for inst in bb.instructions:
                si = getattr(inst, "sync_info", None)
                waits = list(si.on_wait) if si is not None and si.on_wait else []
                if len(waits) > 1:
                    for w in waits[:-1]:
                        new_insts.append(mybir.InstNoOp(
                            name=nc.get_next_instruction_name(), ins=[], outs=[],
                            engine=inst.engine,
                            sync_info=mybir.SyncInfo(on_update=[], on_wait=[w]),
                        ))
                    si.on_wait = [waits[-1]]
                new_insts.append(inst)
            bb.instructions = new_insts


def _build_program(wc):
    W = wc * 128
    nc = bass.Bass(target_bir_lowering=False)

    xabs_r = nc.dram_tensor("xabs_r", [128, 2 * wc], F32, kind="ExternalInput")
    xnb = nc.dram_tensor("xnb", [128, NC_CHUNK], F32, kind="ExternalInput")
    ynb = nc.dram_tensor("ynb", [128, NC_CHUNK], F32, kind="ExternalInput")
    actc = nc.dram_tensor("actc", [128, 2], F32, kind="ExternalInput")
    eye_r = nc.dram_tensor("eye_r", [128, W], MDT, kind="ExternalInput")
    ramp_in = nc.dram_tensor("ramp_in", [RCH * 128, RC_W], MDT, kind="ExternalInput")
    h_winp = nc.dram_tensor("h_winp", [128, W], MDT, kind="ExternalInput")
    wsoc_r = nc.dram_tensor("wsoc_r", [RNN, G * EMB], MDT, kind="ExternalInput")
    wembT = nc.dram_tensor("wembT", [2, EMB], F32, kind="ExternalInput")
    xoffT = nc.dram_tensor("xoffT", [2, NC_CHUNK], F32, kind="ExternalInput")
    b_embsoc = nc.dram_tensor("b_embsoc", [128, 1], F32, kind="ExternalInput")
    wihT = nc.dram_tensor("wihT", [128, 4 * RNN], F32, kind="ExternalInput")
    whhT = nc.dram_tensor("whhT", [RNN, 4 * RNN], F32, kind="ExternalInput")
    bgates_ih = nc.dram_tensor("bgates_ih", [128, 4], F32, kind="ExternalInput")
    bgates_hh = nc.dram_tensor("bgates_hh", [128, 4], F32, kind="ExternalInput")
    hT_c = nc.dram_tensor("hT_c", [RNN, NC_CHUNK], F32, kind="ExternalInput")
    cT_c = nc.dram_tensor("cT_c", [RNN, NC_CHUNK], F32, kind="ExternalInput")
    woutT = nc.dram_tensor("woutT", [RNN, 6 * NMIX], F32, kind="ExternalInput")
    bout = nc.dram_tensor("bout", [6 * NMIX, 1], F32, kind="ExternalInput")
    outT = nc.dram_tensor("outT", [6 * NMIX, NC_CHUNK], F32, kind="ExternalOutput")

    with TileContext(nc) as tc:
        with (
            tc.tile_pool(name="const", bufs=1) as cpool,
            tc.tile_pool(name="masks", bufs=1) as maskpool,
            tc.tile_pool(name="soc", bufs=2) as socpool,
            tc.tile_pool(name="work", bufs=2) as work,
            tc.tile_pool(name="psum", bufs=1, space="PSUM") as pp,
            tc.tile_pool(name="psum_soc", bufs=2, space="PSUM") as pps,
        ):
            # ---- small / latency-critical inputs on the sync queue ----
            xabs_sb = cpool.tile([128, 2 * wc], F32, tag="xabs")
            nc.sync.dma_start(xabs_sb[:, :], xabs_r[:, :])
            xnb_sb = cpool.tile([128, NC_CHUNK], F32, tag="xnb")
            nc.sync.dma_start(xnb_sb[:, :], xnb[:, :])
            ynb_sb = cpool.tile([128, NC_CHUNK], F32, tag="ynb")
            nc.sync.dma_start(ynb_sb[:, :], ynb[:, :])
            actc_sb = cpool.tile([128, 2], F32, tag="actc")
            nc.sync.dma_start(actc_sb[:, :], actc[:, :])
            eye_sb = cpool.tile([128, W], MDT, tag="eye")
            nc.sync.dma_start(eye_sb[:, :], eye_r[:, :])
            xm02 = cpool.tile([128, 2 * wc], F32, tag="xm02")
            nc.vector.tensor_scalar(xm02[:, :], xabs_sb[:, :], 0.2, None,
                                    op0=ALU.add)
            # allocation order fixed (affects SBUF offsets / DVE port
            # behavior); DMA issue order by need-by time.
            h_big = cpool.tile([128, W], MDT, tag="h_big")
            ramp = []
            for c in range(RCH):
                ramp_t = cpool.tile([128, RC_W], MDT, tag=f"ramp{c}")
                ramp.append(ramp_t)
            nc.sync.dma_start(ramp[0][:, :], ramp_in[0:128, :])
            nc.sync.dma_start(h_big[:, :], h_winp[:, :])
            for c in range(1, RCH):
                nc.sync.dma_start(ramp[c][:, :], ramp_in[c * 128:(c + 1) * 128, :])
            wsoc_sb = cpool.tile([RNN, G * EMB], MDT, tag="wsoc")
            nc.scalar.dma_start(wsoc_sb[:, :], wsoc_r[:, :])
            wihT_sb = cpool.tile([128, 4 * RNN], F32, tag="wihT")
            nc.scalar.dma_start(wihT_sb[:, :], wihT[:, :])
            whhT_sb = cpool.tile([RNN, 4 * RNN], F32, tag="whhT")
            nc.scalar.dma_start(whhT_sb[:, :], whhT[:, :])
            woutT_sb = cpool.tile([RNN, 6 * NMIX], F32, tag="woutT")
            nc.scalar.dma_start(woutT_sb[:, :], woutT[:, :])
            wembT_sb = cpool.tile([2, EMB], F32, tag="wembT")
            nc.sync.dma_start(wembT_sb[:, :], wembT[:, :])
            xoffT_sb = cpool.tile([2, NC_CHUNK], F32, tag="xoffT")
            nc.sync.dma_start(xoffT_sb[:, :], xoffT[:, :])
            b_es_sb = cpool.tile([128, 1], F32, tag="b_embsoc")
            nc.sync.dma_start(b_es_sb[:, :], b_embsoc[:, :])
            hT_sb = cpool.tile([RNN, NC_CHUNK], F32, tag="hT")
            nc.sync.dma_start(hT_sb[:, :], hT_c[:, :])
            cT_sb = cpool.tile([RNN, NC_CHUNK], F32, tag="cT")
            nc.sync.dma_start(cT_sb[:, :], cT_c[:, :])
            bgi_sb = cpool.tile([128, 4], F32, tag="bgates_ih")
            nc.sync.dma_start(bgi_sb[:, :], bgates_ih[:, :])
            bgh_sb = cpool.tile([128, 4], F32, tag="bgates_hh")
            nc.sync.dma_start(bgh_sb[:, :], bgates_hh[:, :])
            bout_sb = cpool.tile([6 * NMIX, 1], F32, tag="bout")
            nc.sync.dma_start(bout_sb[:, :], bout[:, :])

            # ---- cell codes per neighbor chunk ----
            # code = 108 - t2x - 11*t2y,  t2 = rint(relu(9 - relu(v + 0.5)))
            # (ACT converts f32->i32 round-to-nearest-even; HW verified)
            cells = []
            for mc in range(wc):
                vx = work.tile([128, NC_CHUNK], F32, tag="vx")
                nc.vector.tensor_scalar(vx[:, :], xnb_sb[:, :],
                                        xm02[:, 2 * mc:2 * mc + 1], -20.0,
                                        op0=ALU.subtract, op1=ALU.mult)
                vy = work.tile([128, NC_CHUNK], F32, tag="vy")
                nc.vector.tensor_scalar(vy[:, :], ynb_sb[:, :],
                                        xm02[:, 2 * mc + 1:2 * mc + 2], -20.0,
                                        op0=ALU.subtract, op1=ALU.mult)
                t2x = work.tile([128, NC_CHUNK], I32, tag="t2x")
                t2y = work.tile([128, NC_CHUNK], I32, tag="t2y")
                if mc == 0:
                    # DVE-only clamp chain for the first chunk: DVE is idle
                    # this early while ACT waits for its table load.
                    for v, t2 in ((vx, t2x), (vy, t2y)):
                        t1 = work.tile([128, NC_CHUNK], F32, tag="t1d")
                        nc.vector.tensor_scalar(t1[:, :], v[:, :], 0.5, 0.0,
                                                op0=ALU.add, op1=ALU.max)
                        t9 = work.tile([128, NC_CHUNK], F32, tag="t9d")
                        nc.vector.tensor_scalar(t9[:, :], t1[:, :], -1.0, 9.0,
                                                op0=ALU.mult, op1=ALU.add)
                        nc.vector.tensor_scalar(t2[:, :], t9[:, :], 0.0, None,
                                                op0=ALU.max)
                else:
                    t1x = work.tile([128, NC_CHUNK], F32, tag="t1x")
                    nc.scalar.activation(t1x[:, :], vx[:, :], ACT.Relu,
                                         bias=actc_sb[:, 0:1], scale=1.0)
                    nc.scalar.activation(t2x[:, :], t1x[:, :], ACT.Relu,
                                         bias=actc_sb[:, 1:2], scale=-1.0)
                    t1y = work.tile([128, NC_CHUNK], F32, tag="t1y")
                    nc.scalar.activation(t1y[:, :], vy[:, :], ACT.Relu,
                                         bias=actc_sb[:, 0:1], scale=1.0)
                    nc.scalar.activation(t2y[:, :], t1y[:, :], ACT.Relu,
                                         bias=actc_sb[:, 1:2], scale=-1.0)
                u = work.tile([128, NC_CHUNK], I32, tag="u")
                nc.vector.tensor_scalar(u[:, :], t2y[:, :], -11, 108,
                                        op0=ALU.mult, op1=ALU.add)
                cc = work.tile([128, NC_CHUNK], MDT, tag=f"cell{mc}")
                nc.vector.tensor_tensor(cc[:, :], u[:, :], t2x[:, :],
                                        op=ALU.subtract)
                nc.vector.tensor_tensor(cc[:, :], cc[:, :],
                                        eye_sb[:, mc * 128:(mc + 1) * 128],
                                        op=ALU.add)
                cells.append(cc)

            # ---- masks: chunk-major so PE group g can start early ----
            masks = {}
            for c in range(RCH):
                for mc in range(wc):
                    m = maskpool.tile([128, RC_W], MDT, tag=f"m{mc}g{c}")
                    cb = cells[mc][:, :].unsqueeze(1).broadcast_to(
                        [128, RC_G, NC_CHUNK])
                    nc.vector.tensor_tensor(m[:, :], cb, ramp[c][:, :],
                                            op=ALU.is_equal)
                    masks[(mc, c)] = m

            # ---- social matmuls + pooling, double-buffered psum groups ----
            xin_ps = pp.tile([128, NC_CHUNK], F32, tag="xin_ps")
            g0 = 0
            for gi, gsz in enumerate(PSG):
                soc_ps = pps.tile([128, 12 * NC_CHUNK], F32, tag="soc_ps")
                for mc in range(wc):
                    for s in range(gsz // 4):
                        cell0 = g0 + s * 4
                        c = cell0 // RC_G
                        off = (cell0 % RC_G) * NC_CHUNK
                        nc.tensor.matmul(
                            soc_ps[:, s * 512:(s + 1) * 512],
                            h_big[:, mc * 128:(mc + 1) * 128],
                            masks[(mc, c)][:, off:off + 512],
                            start=(mc == 0), stop=(mc == wc - 1))
                soc_sb = socpool.tile([128, 12 * NC_CHUNK], MDT, tag="soc_sb")
                nc.scalar.activation(soc_sb[:, :gsz * NC_CHUNK],
                                     soc_ps[:, :gsz * NC_CHUNK], ACT.Copy,
                                     bias=0.0, scale=1.0)
                for gl in range(gsz):
                    g = g0 + gl
                    nc.tensor.matmul(xin_ps[EMB:, :],
                                     wsoc_sb[:, g * EMB:(g + 1) * EMB],
                                     soc_sb[:, gl * NC_CHUNK:(gl + 1) * NC_CHUNK],
                                     start=(g == 0), stop=(g == G - 1))
                g0 += gsz

            # ---- embedding ----
            nc.tensor.matmul(xin_ps[:EMB, :], wembT_sb[:, :], xoffT_sb[:, :],
                             start=True, stop=True)
            xinT = work.tile([128, NC_CHUNK], F32, tag="xinT")
            nc.scalar.activation(xinT[:, :], xin_ps[:, :], ACT.Relu,
                                 bias=b_es_sb[:, 0:1], scale=1.0)

            # ---- LSTM ----
            bg_sb = cpool.tile([128, 4], F32, tag="bgates")
            nc.vector.tensor_tensor(bg_sb[:, :], bgi_sb[:, :], bgh_sb[:, :],
                                    op=ALU.add)
            acts = []
            for q in range(4):
                g_ps = pp.tile([128, NC_CHUNK], F32, tag="g_ps")
                nc.tensor.matmul(g_ps[:, :], wihT_sb[:, q * RNN:(q + 1) * RNN],
                                 xinT[:, :], start=True, stop=False)
                nc.tensor.matmul(g_ps[:, :], whhT_sb[:, q * RNN:(q + 1) * RNN],
                                 hT_sb[:, :], start=False, stop=True)
                gq = work.tile([128, NC_CHUNK], F32, tag=f"gate{q}")
                func = ACT.Tanh if q == 2 else ACT.Sigmoid
                nc.scalar.activation(gq[:, :], g_ps[:, :], func,
                                     bias=bg_sb[:, q:q + 1], scale=1.0)
                acts.append(gq)

            fc = work.tile([128, NC_CHUNK], F32, tag="fc")
            nc.vector.tensor_tensor(fc[:, :], acts[1][:, :], cT_sb[:, :],
                                    op=ALU.mult)
            ig = work.tile([128, NC_CHUNK], F32, tag="ig")
            nc.vector.tensor_tensor(ig[:, :], acts[0][:, :], acts[2][:, :],
                                    op=ALU.mult)
            cnew = work.tile([128, NC_CHUNK], F32, tag="cnew")
            nc.vector.tensor_tensor(cnew[:, :], fc[:, :], ig[:, :], op=ALU.add)
            tc_t = work.tile([128, NC_CHUNK], F32, tag="tc")
            nc.scalar.activation(tc_t[:, :], cnew[:, :], ACT.Tanh,
                                 bias=0.0, scale=1.0)
            hn = work.tile([128, NC_CHUNK], F32, tag="hn")
            nc.vector.tensor_tensor(hn[:, :], acts[3][:, :], tc_t[:, :],
                                    op=ALU.mult)

            # ---- output projection ----
            out_ps = pp.tile([6 * NMIX, NC_CHUNK], F32, tag="g_ps")
            nc.tensor.matmul(out_ps[:, :], woutT_sb[:, :], hn[:, :],
                             start=True, stop=True)
            outT_sb = work.tile([6 * NMIX, NC_CHUNK], F32, tag="outT")
            nc.vector.tensor_scalar(outT_sb[:, :], out_ps[:, :],
                                    bout_sb[:, 0:1], None, op0=ALU.add)
            nc.sync.dma_start(outT[:, :], outT_sb[:, :])

    _split_multi_waits(nc)
    return nc


_NC_CACHE = {}


def _get_program(wc):
    if wc not in _NC_CACHE:
        _NC_CACHE[wc] = _build_program(wc)
    return _NC_CACHE[wc]


def _make_ramp():
    gy, gx, n = np.meshgrid(np.arange(GS), np.arange(GS), np.arange(NC_CHUNK),
                            indexing="ij")
    vals = (12 + gx + 11 * gy).reshape(1, G * NC_CHUNK)
    full = np.broadcast_to(vals, (128, G * NC_CHUNK)).astype(MNP)
    # chunked layout: [RCH*128, RC_W], chunk c = rows 128c..128c+127
    return np.ascontiguousarray(
        full.reshape(128, RCH, RC_W).transpose(1, 0, 2).reshape(RCH * 128, RC_W))


def _prep_inputs(xoff, xabs, h0, c0, W_emb, b_emb, W_soc, b_soc,
                 W_ih, W_hh, b_ih, b_hh, W_out, b_out):
    f32 = np.float32
    xoff = np.asarray(xoff, f32)
    xabs = np.asarray(xabs, f32)
    h = np.asarray(h0, f32)[0]
    c = np.asarray(c0, f32)[0]
    W_emb = np.asarray(W_emb, f32)
    W_soc = np.asarray(W_soc, f32)
    W_ih = np.asarray(W_ih, f32)
    W_hh = np.asarray(W_hh, f32)
    W_out = np.asarray(W_out, f32)

    perm = np.argsort(xabs[:, 0], kind="stable")
    xs = xabs[perm]
    xoff_s = xoff[perm]
    h_s = h[perm]
    c_s = c[perm]

    los, his = [], []
    for k in range(NCORE):
        ch = xs[k * NC_CHUNK:(k + 1) * NC_CHUNK, 0]
        los.append(np.searchsorted(xs[:, 0], ch.min() - f32(0.21), "left"))
        his.append(np.searchsorted(xs[:, 0], ch.max() + f32(0.21), "right"))
    W = int(max(hh - l for l, hh in zip(los, his)))
    W = max(128, -(-W // 128) * 128)
    wc = W // 128
    lo = [min(max(0, l), N - W) for l in los]

    h_b = h_s.astype(MNP)
    wsoc_r = np.ascontiguousarray(
        W_soc.reshape(EMB, G, RNN).transpose(2, 1, 0).reshape(RNN, G * EMB)
    ).astype(MNP)
    wembT = np.ascontiguousarray(W_emb.T)
    b_embsoc = np.ascontiguousarray(
        np.concatenate([np.asarray(b_emb, f32), np.asarray(b_soc, f32)])[:, None])
    wihT = np.ascontiguousarray(W_ih.T)
    whhT = np.ascontiguousarray(W_hh.T)
    bgates_ih = np.ascontiguousarray(np.asarray(b_ih, f32).reshape(4, RNN).T)
    bgates_hh = np.ascontiguousarray(np.asarray(b_hh, f32).reshape(4, RNN).T)
    woutT = np.ascontiguousarray(W_out.T)
    bout = np.ascontiguousarray(np.asarray(b_out, f32)[:, None])
    ramp = _make_ramp()
    actc = np.ascontiguousarray(
        np.broadcast_to(np.array([0.5, 9.0], f32)[None, :], (128, 2)))

    in_maps = []
    for k in range(NCORE):
        sl = slice(k * NC_CHUNK, (k + 1) * NC_CHUNK)
        win = slice(lo[k], lo[k] + W)
        eye_r = np.zeros((128, W), MNP)
        idx = np.arange(128)
        ms = k * NC_CHUNK + idx - lo[k]
        eye_r[ms % 128, (ms // 128) * 128 + idx] = MNP(1000.0)
        xw = xs[win]
        hw = h_b[win]
        in_maps.append({
            "xabs_r": np.ascontiguousarray(
                xw.reshape(wc, 128, 2).transpose(1, 0, 2).reshape(128, 2 * wc)),
            "xnb": np.ascontiguousarray(
                np.broadcast_to(xs[sl, 0][None, :], (128, NC_CHUNK))),
            "ynb": np.ascontiguousarray(
                np.broadcast_to(xs[sl, 1][None, :], (128, NC_CHUNK))),
            "actc": actc,
            "eye_r": eye_r,
            "ramp_in": ramp,
            "h_winp": np.ascontiguousarray(
                hw.reshape(wc, 128, RNN).transpose(1, 0, 2).reshape(128, W)),
            "wsoc_r": wsoc_r,
            "wembT": wembT,
            "xoffT": np.ascontiguousarray(xoff_s[sl].T),
            "b_embsoc": b_embsoc,
            "wihT": wihT,
            "whhT": whhT,
            "bgates_ih": bgates_ih,
            "bgates_hh": bgates_hh,
            "hT_c": np.ascontiguousarray(h_s[sl].T),
            "cT_c": np.ascontiguousarray(c_s[sl].T),
            "woutT": woutT,
            "bout": bout,
        })
    return in_maps, perm, wc


def kernel(**inputs):
    in_maps, perm, wc = _prep_inputs(**inputs)
    nc = _get_program(wc)
    res = run_bass_kernel_spmd(nc, in_maps, list(range(NCORE)))
    outT = np.concatenate([res.results[k]["outT"] for k in range(NCORE)],
                          axis=1)
    out_sorted = outT.T
    out = np.empty_like(out_sorted)
    out[perm] = out_sorted
    return tuple(np.ascontiguousarray(out[:, i * NMIX:(i + 1) * NMIX])
                 for i in range(6))
